# revision 13
# baseline (speedup 1.0000x reference)
"""PlasticLinear Trainium2 kernel.

Computes, for B=128, OUT=IN=512 (fp32):
    eff      = weight + plastic_scale * hebb          (per-sample weights)
    out_pre  = einsum('bi,boi->bo', input, eff)
    hebb_new = hebb + plastic_lr * a * (in - a*hebb)  (a = out_pre, Oja rule)
    out      = out_pre + bias

Sharding: batch is split across 8 NeuronCores (16 samples each); all
parameter tensors are replicated. Per core, per sample b (loop order is
b-outer so dependency chains are short), with hebb[b] held as four
[128o x 512i] tiles (one 1MB contiguous DMA):
  phase 1:  u_all = S * R_b in one FD=2048 DVE pass (R_b = input row
    broadcast across partitions), then per oc a fused
    scalar_tensor_tensor pass computes hebb*u with a free-dim sum ->
    s_h[o]; a = s_h + (W @ in)[o,b] (small PE matmul, W^T from host).
  phase 2 (Oja update), per oc: two fused STT passes
        E = in - a*hebb      (a is a per-partition scalar)
        w = (E * a) * lr
    and the final hebb_new = hebb + w is done on the otherwise-idle
    TensorEngine as two identity matmuls accumulated in PSUM, then
    evacuated by the ScalarEngine. fp32 matmuls are ~4x slower per
    column than bf16 so the PE carries only this one accumulation.
The VectorEngine is the bottleneck at 4 full passes over the data.
"""

import numpy as np
from contextlib import ExitStack

import concourse.bass as bass
import concourse.tile as tile
from concourse import bacc, mybir
from concourse.alu_op_type import AluOpType
from concourse.bass_utils import run_bass_kernel_spmd

B, IN, OUT = 128, 512, 512
NCORES = 8
BS = B // NCORES           # 16 samples per core
OC = OUT // 128            # 4 output-row chunks
F32 = mybir.dt.float32

_CACHE = {}


def _build(reps=1):
    nc = bacc.Bacc("TRN2", target_bir_lowering=False, debug=False)

    hebb = nc.dram_tensor("hebb", [BS, OUT, IN], F32, kind="ExternalInput")
    inp = nc.dram_tensor("inp", [BS, IN], F32, kind="ExternalInput")
    inT = nc.dram_tensor("inT", [IN, BS], F32, kind="ExternalInput")
    S = nc.dram_tensor("pscale", [OUT, IN], F32, kind="ExternalInput")
    LR = nc.dram_tensor("plr", [OUT, IN], F32, kind="ExternalInput")
    WT = nc.dram_tensor("wT", [IN, OUT], F32, kind="ExternalInput")
    IDN = nc.dram_tensor("idn", [128, 128], F32, kind="ExternalInput")
    BCOL = nc.dram_tensor("bias_col", [OUT], F32, kind="ExternalInput")
    out = nc.dram_tensor("out", [BS, OUT], F32, kind="ExternalOutput")
    hnew = nc.dram_tensor("hebb_new", [BS, OUT, IN], F32, kind="ExternalOutput")

    with ExitStack() as ctx:
        tc = ctx.enter_context(tile.TileContext(nc))
        consts = ctx.enter_context(tc.tile_pool(name="consts", bufs=1))
        hpool = ctx.enter_context(tc.tile_pool(name="h", bufs=6))
        upool = ctx.enter_context(tc.tile_pool(name="u", bufs=3))
        scpool = ctx.enter_context(tc.tile_pool(name="scr", bufs=3))
        wpool = ctx.enter_context(tc.tile_pool(name="w", bufs=4))
        npool = ctx.enter_context(tc.tile_pool(name="new", bufs=3))
        epool = ctx.enter_context(tc.tile_pool(name="e", bufs=4))
        smalls = ctx.enter_context(tc.tile_pool(name="smalls", bufs=3))
        psn = ctx.enter_context(tc.tile_pool(name="psn", bufs=4, space="PSUM"))
        psmall = ctx.enter_context(tc.tile_pool(name="psm", bufs=2, space="PSUM"))

        dma = nc.sync

        # first two hebb tiles + their broadcast rows go first so the
        # vector engine can start as soon as s_all lands
        hb_pre = {}
        for b in range(2):
            hbt = hpool.tile([128, OC, IN], F32, tag="h", name=f"hb_pre{b}")
            dma.dma_start(out=hbt, in_=hebb[b].rearrange("(c p) i -> p c i", p=128))
            hb_pre[b] = hbt
        s_all = consts.tile([128, OC, IN], F32, tag="s_all")
        dma.dma_start(out=s_all, in_=S.rearrange("(c p) i -> p c i", p=128))
        lr_t = [consts.tile([128, IN], F32, tag=f"lr{c}", name=f"lr{c}") for c in range(OC)]
        wt_t = [consts.tile([128, OUT], F32, tag=f"wt{c}", name=f"wt{c}") for c in range(OC)]
        int_t = [consts.tile([128, BS], F32, tag=f"it{c}", name=f"it{c}") for c in range(OC)]
        for c in range(OC):
            sl = slice(c * 128, (c + 1) * 128)
            dma.dma_start(out=lr_t[c], in_=LR[sl, :])
            dma.dma_start(out=wt_t[c], in_=WT[sl, :])
            dma.dma_start(out=int_t[c], in_=inT[sl, :])
        idn_t = consts.tile([128, 128], F32, tag="idn")
        dma.dma_start(out=idn_t, in_=IDN[:, :])
        bcol_t = consts.tile([128, OC], F32, tag="bcol")
        dma.dma_start(out=bcol_t, in_=BCOL.rearrange("(c p) -> p c", p=128))

        # input rows replicated across all 128 partitions: r_t[b] == inp[b, :]
        r_t = []
        for b in range(BS):
            rt = consts.tile([128, IN], F32, tag=f"r{b}", name=f"r{b}")
            row = inp[b : b + 1, :]
            bcast = bass.AP(
                tensor=row.tensor, offset=row.offset, ap=[[0, 128], row.ap[1]]
            )
            nc.gpsimd.dma_start(out=rt, in_=bcast)
            r_t.append(rt)

        # OW[o, b] = sum_i W[o, i] * in[b, i]  (all oc chunks in one PSUM bank)
        ow_ps = psmall.tile([128, OC * BS], F32, tag="ow")
        for oc in range(OC):
            for ic in range(OC):
                nc.tensor.matmul(
                    out=ow_ps[:, oc * BS : (oc + 1) * BS],
                    lhsT=wt_t[ic][:, oc * 128 : (oc + 1) * 128],
                    rhs=int_t[ic],
                    start=(ic == 0),
                    stop=(ic == OC - 1),
                )
        ow_sb = consts.tile([128, OC * BS], F32, tag="ow_sb")
        nc.scalar.copy(ow_sb, ow_ps)

        for rep in range(reps):
            for b in range(BS):
                if rep == 0 and b in hb_pre:
                    hb = hb_pre.pop(b)
                else:
                    hb = hpool.tile([128, OC, IN], F32, tag="h")
                    dma.dma_start(
                        out=hb, in_=hebb[b].rearrange("(c p) i -> p c i", p=128)
                    )
                rb = r_t[b][:, :]
                # R_b broadcast along the oc free dim (step 0)
                rb4 = bass.AP(
                    tensor=rb.tensor,
                    offset=rb.offset,
                    ap=[rb.ap[0], [0, OC], rb.ap[1]],
                )
                ua = upool.tile([128, OC, IN], F32, tag="u")
                nc.vector.tensor_tensor(ua, s_all, rb4, AluOpType.mult)
                acc = smalls.tile([128, OC], F32, tag="acc")
                for oc in range(OC):
                    sc = scpool.tile([128, IN], F32, tag="scr")
                    # hebb*u with free-dim sum into acc — one DVE pass
                    nc.vector.scalar_tensor_tensor(
                        out=sc,
                        in0=hb[:, oc, :],
                        scalar=1.0,
                        in1=ua[:, oc, :],
                        op0=AluOpType.mult,
                        op1=AluOpType.mult,
                        accum_out=acc[:, oc : oc + 1],
                    )
                # a[:, oc] = s_h + (W@in); ow_sb columns are oc*BS+b
                ow_b = bass.AP(
                    tensor=ow_sb.tensor,
                    offset=ow_sb.offset + b,
                    ap=[ow_sb.ap[0], [BS, OC]],
                )
                a_b = smalls.tile([128, OC], F32, tag="a_b")
                nc.vector.tensor_add(a_b, acc, ow_b)
                na_b = smalls.tile([128, OC], F32, tag="na_b")
                nc.vector.tensor_scalar(na_b, a_b, -1.0, None, AluOpType.mult)

                nt = npool.tile([128, OC, IN], F32, tag="new")
                for oc in range(OC):
                    # E = in - a*hebb   (one STT pass; a is per-partition)
                    et = epool.tile([128, IN], F32, tag="e")
                    nc.vector.scalar_tensor_tensor(
                        out=et,
                        in0=hb[:, oc, :],
                        scalar=na_b[:, oc : oc + 1],
                        in1=rb,
                        op0=AluOpType.mult,
                        op1=AluOpType.add,
                    )
                    # w = (E * a) * lr = lr * a * (in - a*hebb)
                    wt_ = wpool.tile([128, IN], F32, tag="w")
                    nc.vector.scalar_tensor_tensor(
                        out=wt_,
                        in0=et,
                        scalar=a_b[:, oc : oc + 1],
                        in1=lr_t[oc],
                        op0=AluOpType.mult,
                        op1=AluOpType.mult,
                    )
                    # hebb_new = hebb + w via PSUM accumulation on the PE
                    ps_n = psn.tile([128, IN], F32, tag="psn")
                    nc.tensor.matmul(
                        out=ps_n, lhsT=idn_t, rhs=hb[:, oc, :], start=True, stop=False
                    )
                    nc.tensor.matmul(out=ps_n, lhsT=idn_t, rhs=wt_, start=False, stop=True)
                    nc.scalar.copy(nt[:, oc, :], ps_n)
                dma.dma_start(
                    out=hnew[b].rearrange("(c p) i -> p c i", p=128), in_=nt
                )

                # out[b, :] = a + bias: add bias (bcol[p, oc] = bias[oc*128+p]),
                # transpose [128, OC] -> [OC, 128] on the PE, write one row.
                ab2 = smalls.tile([128, OC], F32, tag="ab2")
                nc.vector.tensor_add(ab2, a_b, bcol_t)
                ps_t = psmall.tile([OC, 128], F32, tag="pst")
                nc.tensor.transpose(ps_t, ab2, idn_t)
                orow = smalls.tile([OC, 128], F32, tag="orow")
                nc.scalar.copy(orow, ps_t)
                dma.dma_start(
                    out=out[b].rearrange("(c j) -> c j", j=128), in_=orow
                )

    nc.compile()
    return nc


def _get_nc():
    if "nc" not in _CACHE:
        _CACHE["nc"] = _build()
    return _CACHE["nc"]


def build_in_maps(input, hebb, weight, plastic_scale, plastic_lr, bias):
    input = np.ascontiguousarray(input, dtype=np.float32)
    hebb = np.ascontiguousarray(hebb, dtype=np.float32)
    weight = np.ascontiguousarray(weight, dtype=np.float32)
    plastic_scale = np.ascontiguousarray(plastic_scale, dtype=np.float32)
    plastic_lr = np.ascontiguousarray(plastic_lr, dtype=np.float32)
    bias = np.ascontiguousarray(bias, dtype=np.float32)

    wT = np.ascontiguousarray(weight.T)
    idn = np.eye(128, dtype=np.float32)


    in_maps = []
    for c in range(NCORES):
        bsl = slice(c * BS, (c + 1) * BS)
        in_shard = np.ascontiguousarray(input[bsl])
        in_maps.append(
            {
                "hebb": np.ascontiguousarray(hebb[bsl]),
                "inp": in_shard,
                "inT": np.ascontiguousarray(in_shard.T),
                "pscale": plastic_scale,
                "plr": plastic_lr,
                "wT": wT,
                "idn": idn,
                "bias_col": bias,
            }
        )
    return in_maps


def kernel(input, hebb, weight, plastic_scale, plastic_lr, bias):
    in_maps = build_in_maps(input, hebb, weight, plastic_scale, plastic_lr, bias)
    nc = _get_nc()
    res = run_bass_kernel_spmd(nc, in_maps, core_ids=list(range(NCORES)))
    out = np.concatenate([r["out"] for r in res.results], axis=0)
    hebb_new = np.concatenate([r["hebb_new"] for r in res.results], axis=0)
    return out, hebb_new
